# revision 22
# baseline (speedup 1.0000x reference)
"""AMRPA attention wrapper kernel for 8 TRN2 NeuronCores.

Sharding: data-parallel over (batch, seq-half). Core c handles batch b=c//2,
query rows [h*1024, (h+1)*1024) with h=c%2. Each core computes k/v for its
full batch (duplicated across the pair) and its own half of the query rows;
outputs are concatenated on host. No collectives.

SPMD trick: all cores run one graph that reads query columns [0, 1024) of
hsT; the host rolls hsT's sequence axis (and paT's key axis identically) so
each core's query rows land there. Key order is permuted consistently in
kT/v/paT, and every contraction over keys is permutation-invariant.

Math (per core, Sq=1024 query rows, S=2048 keys, H=1024):
  qT = Wq^T hs^T, kT = (Wk/sqrt(H))^T hs^T, v = hs Wv           (T-major)
  g = sigmoid(q . w_gate)                                        (per row)
  mvT = v^T paT  (contraction over keys; paT = pa^T)
  tfT = (Wm e^-0.5)^T mvT
  qhatT = qT + g * tfT      (folds the memory bias into the query)
  logits = qhat kT          (scores + memory_bias in one matmul)
  probs = exp(logits)       (logits are O(1) -- no max subtraction needed)
  context = (probs v) / rowsum(probs)

All matmul operands bf16 (fp32 PSUM accumulation); scale factors folded into
the weights on host.
"""

import math

import numpy as np
import ml_dtypes

import concourse.bass as bass
import concourse.mybir as mybir
import concourse.tile as tile
from concourse.bass_utils import run_bass_kernel_spmd
from concourse.masks import make_identity
from concourse.vector_clock import ScopedClock

BF16 = mybir.dt.bfloat16
F32 = mybir.dt.float32
FP8 = mybir.dt.float8e4

# fp8 scale folding for the memory-bias chain: pa is scaled by PA_SCALE and
# Wm by WM_SCALE on host (to sit in fp8 e4m3's normal range); the combined
# factor is divided back out of the gate broadcast.
PA_SCALE = 512.0
WM_SCALE = 16.0

B, S, H = 4, 2048, 1024
SQ = S // 2  # query rows per core
N_CORES = 8
NT_H = H // 128   # 8 partition tiles over hidden dim
NT_S = S // 128   # 16 partition tiles over sequence
NT_Q = SQ // 128  # 8 query row tiles per core
NC_S = S // 512   # 4 free-dim chunks over sequence
NC_Q = SQ // 512  # 2 free-dim chunks over query rows
NC_H = H // 512   # 2 free-dim chunks over hidden

# ---------------------------------------------------------------------------
# Workaround: this walrus build allows only one sync-wait on a Drain
# instruction; Tile's kernel-tail drain carries one wait per DMA-HW
# semaphore. Split the tail drain into a chain of single-wait drains.
# ---------------------------------------------------------------------------


def _patched_drain_and_barrier(self, tick_clock, wait_clock):
    nc = self.nc
    drain_inst = nc.sync.drain()
    wait_clock.add_sem_waits(
        drain_inst.ins, ScopedClock({None: tick_clock.global_clock})
    )
    si = drain_inst.ins.sync_info
    if si is not None and si.on_wait and len(si.on_wait) > 1:
        waits = list(si.on_wait)
        si.on_wait = waits[:1]
        for w in waits[1:]:
            d = nc.sync.drain()
            dsi = d.ins.sync_info
            if dsi is None:
                d.ins.sync_info = mybir.SyncInfo(on_wait=[w], on_update=[])
            else:
                dsi.on_wait = [w]

    nc.all_engine_barrier()
    assert self.sems is not None
    popped = nc._tile_sem_poison_stack.pop()
    assert popped is self._sem_poison
    nc.clear_and_free_semaphores(list(self.sems.allocated().values()))
    nc.all_engine_barrier()


tile.TileContext._drain_and_barrier = _patched_drain_and_barrier


def _split_multi_wait_instructions(nc: bass.Bass):
    """Walrus here allows only one sync-wait per instruction. Move extra
    waits onto injected same-engine NoOps placed just before the owner."""
    bbs = [(bb, list(bb.instructions)) for f in nc.m.functions for bb in f.blocks]
    new_lists = []
    for bb, insts in bbs:
        new_list = []
        for inst in insts:
            si = inst.sync_info
            if si is not None and si.on_wait and len(si.on_wait) > 1:
                waits = list(si.on_wait)
                for w in waits[:-1]:
                    bi = nc.engines[inst.engine].nop(nofuse=True)
                    ni = bi.ins
                    ni.sync_info = mybir.SyncInfo(on_wait=[w], on_update=[])
                    new_list.append(ni)
                si.on_wait = [waits[-1]]
            new_list.append(inst)
        new_lists.append((bb, new_list))
    for bb, nl in new_lists:
        bb.instructions = nl


def build_nc() -> bass.Bass:
    nc = bass.Bass()

    hsT_ext = nc.declare_dram_parameter("hsT", [H, S], BF16, isOutput=False)
    hsq_ext = nc.declare_dram_parameter("hsq", [H, SQ], BF16, isOutput=False)
    pa8_ext = nc.declare_dram_parameter("pa8", [NT_S // 2, 128, 2, SQ], FP8, isOutput=False)
    wq_ext = nc.declare_dram_parameter("wq", [H, H], BF16, isOutput=False)
    # wk/wv carry only this core's output-column half; the other half of
    # kT / v comes from the pair AllGather below
    wk_ext = nc.declare_dram_parameter("wk", [H, H // 2], BF16, isOutput=False)
    wv_ext = nc.declare_dram_parameter("wv", [H, H // 2], BF16, isOutput=False)
    wm_ext = nc.declare_dram_parameter("wm", [NT_H // 2, 128, 2, H], FP8, isOutput=False)
    wg_ext = nc.declare_dram_parameter("wg", [128, NT_H], BF16, isOutput=False)
    out_ext = nc.declare_dram_parameter("out", [SQ, H], F32, isOutput=True)

    PAIR_GROUPS = [[2 * i, 2 * i + 1] for i in range(N_CORES // 2)]
    DR = mybir.MatmulPerfMode.DoubleRow

    MULT = mybir.AluOpType.mult
    ADD = mybir.AluOpType.add

    with tile.TileContext(nc) as tc:
        with (
            tc.tile_pool(name="persist", bufs=1) as pp,
            tc.tile_pool(name="dram_cc", bufs=1, space="DRAM") as dcc,
        ):
            # small constants
            identity = pp.tile([128, 128], BF16)
            make_identity(nc, identity)
            ones_row = pp.tile([1, 128], BF16)
            nc.vector.memset(ones_row, 1.0)
            wg_sb = pp.tile([128, NT_H], BF16)
            nc.sync.dma_start(out=wg_sb, in_=wg_ext[:, :])

            qT_sb = [pp.tile([128, SQ], BF16, name=f"qT{t}") for t in range(NT_H)]
            v_big = pp.tile([128, NT_S * H], BF16)
            v3d = v_big.rearrange("p (st d) -> p st d", d=H)
            v8_sb = [pp.tile([128, 2, H], FP8, name=f"v8_{t}") for t in range(NT_S // 2)]
            qhatT_sb = [
                pp.tile([128, SQ], BF16, name=f"qhatT{t}") for t in range(NT_H)
            ]
            pa8_sb = [pp.tile([128, 2, SQ], FP8, name=f"pa8_{t}") for t in range(NT_S // 2)]
            wm8_sb = [pp.tile([128, 2, H], FP8, name=f"wm8_{t}") for t in range(NT_H // 2)]
            g_bcast = pp.tile([128, SQ], BF16)
            g_row = pp.tile([1, SQ], BF16)
            rsum_sb = [pp.tile([128, 1], F32, name=f"rsum{t}") for t in range(NT_Q)]

            vb_in = dcc.tile([S, H // 2], BF16)
            vb_out = dcc.tile([2 * S, H // 2], BF16)
            kb_in = dcc.tile([H // 2, S], BF16)
            kb_out = dcc.tile([H, S], BF16)

            # one PSUM pool across stages 1-2 so the fp8 memory-chain matmuls
            # enter the same accumulator rotation with no pool barrier
            with tc.tile_pool(name="ps12", bufs=4, space="PSUM") as ps1:
                # ---- stage 1: projections + gates + pair AllGather ----
                with tc.tile_pool(name="stage1", bufs=1) as s1:
                    hsT_sb = [
                        s1.tile([128, S], BF16, name=f"hsT{t}") for t in range(NT_H)
                    ]
                    wq_sb = [
                        s1.tile([128, H], BF16, name=f"wqs{t}") for t in range(NT_H)
                    ]
                    wk_sb = [
                        s1.tile([128, H // 2], BF16, name=f"wks{t}") for t in range(NT_H)
                    ]
                    wv_sb = [
                        s1.tile([128, H // 2], BF16, name=f"wvs{t}") for t in range(NT_H)
                    ]
                    hsq_sb = [
                        s1.tile([128, SQ], BF16, name=f"hsq{t}") for t in range(NT_H)
                    ]
                    # hsT + wv first: the v matmuls are the first consumers
                    for t in range(NT_H):
                        nc.sync.dma_start(out=hsT_sb[t], in_=hsT_ext[t * 128:(t + 1) * 128, :])
                        nc.sync.dma_start(out=wv_sb[t], in_=wv_ext[t * 128:(t + 1) * 128, :])
                    for t in range(NT_H):
                        nc.sync.dma_start(out=wk_sb[t], in_=wk_ext[t * 128:(t + 1) * 128, :])
                    for t in range(NT_H):
                        nc.sync.dma_start(out=wq_sb[t], in_=wq_ext[t * 128:(t + 1) * 128, :])
                        nc.sync.dma_start(out=hsq_sb[t], in_=hsq_ext[t * 128:(t + 1) * 128, :])
                    for t in range(NT_S // 2):
                        nc.sync.dma_start(out=pa8_sb[t], in_=pa8_ext[t])
                    for t in range(NT_H // 2):
                        nc.sync.dma_start(out=wm8_sb[t], in_=wm_ext[t])

                    # v half: v[s, d_local] = sum_hi hsT[hi, s] Wv_half[hi, :]
                    for st in range(NT_S):
                        acc = ps1.tile([128, 512], F32, tag="acc")
                        for hi in range(NT_H):
                            nc.tensor.matmul(
                                acc,
                                hsT_sb[hi][:, st * 128:(st + 1) * 128],
                                wv_sb[hi],
                                start=(hi == 0),
                                stop=(hi == NT_H - 1),
                            )
                        nc.any.tensor_copy(
                            out=v_big[:, st * H:st * H + 512], in_=acc
                        )
                    nc.gpsimd.dma_start(
                        out=vb_in.rearrange("(st p) d -> p st d", p=128),
                        in_=v3d[:, :, 0:512],
                    )
                    nc.gpsimd.collective_compute(
                        "AllGather",
                        mybir.AluOpType.bypass,
                        replica_groups=PAIR_GROUPS,
                        ins=[vb_in.opt()],
                        outs=[vb_out.opt()],
                    )
                    nc.gpsimd.dma_start(
                        out=v3d[:, :, 0:512],
                        in_=vb_out[0:S].rearrange("(st p) d -> p st d", p=128),
                    )
                    nc.gpsimd.dma_start(
                        out=v3d[:, :, 512:1024],
                        in_=vb_out[S:2 * S].rearrange("(st p) d -> p st d", p=128),
                    )

                    # kT half: staged straight to the DRAM bounce buffer; the
                    # full kT is gathered into SBUF lazily in stage 3
                    for ho in range(NT_H // 2):
                        for sc in range(NC_S):
                            acc = ps1.tile([128, 512], F32, tag="acc")
                            for hi in range(NT_H):
                                nc.tensor.matmul(
                                    acc,
                                    wk_sb[hi][:, ho * 128:(ho + 1) * 128],
                                    hsT_sb[hi][:, sc * 512:(sc + 1) * 512],
                                    start=(hi == 0),
                                    stop=(hi == NT_H - 1),
                                )
                            kstg = s1.tile([128, 512], BF16, tag="kstg", bufs=4)
                            nc.any.tensor_copy(out=kstg, in_=acc)
                            nc.sync.dma_start(
                                out=kb_in[ho * 128:(ho + 1) * 128, sc * 512:(sc + 1) * 512],
                                in_=kstg,
                            )
                    nc.gpsimd.collective_compute(
                        "AllGather",
                        mybir.AluOpType.bypass,
                        replica_groups=PAIR_GROUPS,
                        ins=[kb_in.opt()],
                        outs=[kb_out.opt()],
                    )

                    # qT over this core's query rows (hsq input)
                    for ho in range(NT_H):
                        for qc in range(NC_Q):
                            acc = ps1.tile([128, 512], F32, tag="acc")
                            for hi in range(NT_H):
                                nc.tensor.matmul(
                                    acc,
                                    wq_sb[hi][:, ho * 128:(ho + 1) * 128],
                                    hsq_sb[hi][:, qc * 512:(qc + 1) * 512],
                                    start=(hi == 0),
                                    stop=(hi == NT_H - 1),
                                )
                            nc.any.tensor_copy(out=qT_sb[ho][:, qc * 512:(qc + 1) * 512], in_=acc)

                    # gates: glog[1, q] = sum_t wg[:, t]^T qT[t]; sigmoid; bcast
                    for qc in range(NC_Q):
                        gacc = ps1.tile([1, 512], F32, tag="gacc", bufs=2)
                        for t in range(NT_H):
                            nc.tensor.matmul(
                                gacc,
                                wg_sb[:, t:t + 1],
                                qT_sb[t][:, qc * 512:(qc + 1) * 512],
                                start=(t == 0),
                                stop=(t == NT_H - 1),
                            )
                        nc.scalar.activation(
                            g_row[:, qc * 512:(qc + 1) * 512],
                            gacc,
                            mybir.ActivationFunctionType.Sigmoid,
                        )
                    for qc in range(NC_Q):
                        gb = ps1.tile([128, 512], F32, tag="gb", bufs=2)
                        nc.tensor.matmul(
                            gb,
                            ones_row,
                            g_row[:, qc * 512:(qc + 1) * 512],
                            start=True,
                            stop=True,
                        )
                        nc.vector.tensor_scalar_mul(
                            g_bcast[:, qc * 512:(qc + 1) * 512],
                            gb,
                            1.0 / (PA_SCALE * WM_SCALE),
                        )

                    # fp8 copy of v for the memory-bias chain, emitted last so
                    # stage-1 copies aren't queued behind the collective wait
                    for kp in range(NT_S // 2):
                        nc.any.tensor_copy(
                            out=v8_sb[kp].rearrange("p a b -> p (a b)"),
                            in_=v_big[:, (2 * kp) * H:(2 * kp + 2) * H],
                        )

                # ---- stage 2: fp8 DoubleRow memory chain ----
                with tc.tile_pool(name="stage2", bufs=1) as s2:
                    mv8_sb = [
                        s2.tile([128, 2, SQ], FP8, name=f"mv8_{t}")
                        for t in range(NT_H // 2)
                    ]
                    # mvT[d, q] = sum_k v[k, d] paT[k, q]  (fp8, K=256/instr)
                    for qc in range(NC_Q):
                        for d in range(NT_H):
                            acc = ps1.tile([128, 512], F32, tag="acc")
                            for kp in range(NT_S // 2):
                                nc.tensor.matmul(
                                    acc,
                                    v8_sb[kp][:, :, d * 128:(d + 1) * 128],
                                    pa8_sb[kp][:, :, qc * 512:(qc + 1) * 512],
                                    perf_mode=DR,
                                    start=(kp == 0),
                                    stop=(kp == NT_S // 2 - 1),
                                )
                            nc.any.tensor_copy(
                                out=mv8_sb[d // 2][:, d % 2, qc * 512:(qc + 1) * 512],
                                in_=acc,
                            )

                    # tfT[do, q] = sum_d Wm[d, do] mvT[d, q];
                    # qhatT = qT + (g_bcast / (PA_SCALE*WM_SCALE)) * tfT
                    for qc in range(NC_Q):
                        for do in range(NT_H):
                            acc = ps1.tile([128, 512], F32, tag="acc")
                            for dp in range(NT_H // 2):
                                nc.tensor.matmul(
                                    acc,
                                    wm8_sb[dp][:, :, do * 128:(do + 1) * 128],
                                    mv8_sb[dp][:, :, qc * 512:(qc + 1) * 512],
                                    perf_mode=DR,
                                    start=(dp == 0),
                                    stop=(dp == NT_H // 2 - 1),
                                )
                            sl = slice(qc * 512, (qc + 1) * 512)
                            tmp = s2.tile([128, 512], BF16, tag="gm_tmp", bufs=3)
                            nc.vector.tensor_tensor(tmp, acc, g_bcast[:, sl], MULT)
                            nc.vector.tensor_tensor(
                                qhatT_sb[do][:, sl], tmp, qT_sb[do][:, sl], ADD
                            )

            # ---- stage 3: per q-tile attention ----
            # kT arrives here straight from the AllGather bounce buffer
            with (
                tc.tile_pool(name="stage3", bufs=1) as s3,
                tc.tile_pool(name="ps_logit", bufs=2, space="PSUM") as pslg,
                tc.tile_pool(name="ps_small", bufs=2, space="PSUM") as pssm,
            ):
                kT_sb = [s3.tile([128, S], BF16, name=f"kT{t}") for t in range(NT_H)]
                for t in range(NT_H):
                    nc.sync.dma_start(
                        out=kT_sb[t], in_=kb_out[t * 128:(t + 1) * 128, :]
                    )
                for qt in range(NT_Q):
                    qsl = slice(qt * 128, (qt + 1) * 128)
                    probs = s3.tile([128, S], BF16, tag="probs", bufs=2)
                    hsum = [None, None]
                    for half in range(2):
                        lg = pslg.tile([128, 1024], F32, tag="lg")
                        for kk2 in range(2):
                            kk = half * 2 + kk2
                            for d in range(NT_H):
                                nc.tensor.matmul(
                                    lg[:, kk2 * 512:(kk2 + 1) * 512],
                                    qhatT_sb[d][:, qsl],
                                    kT_sb[d][:, kk * 512:(kk + 1) * 512],
                                    start=(d == 0),
                                    stop=(d == NT_H - 1),
                                )
                        hs_t = s3.tile(
                            [128, 1], F32, tag=f"hsum{half}", bufs=2, name=f"hs{half}"
                        )
                        nc.scalar.activation(
                            probs[:, half * 1024:(half + 1) * 1024],
                            lg,
                            mybir.ActivationFunctionType.Exp,
                            accum_out=hs_t,
                        )
                        hsum[half] = hs_t
                    nc.vector.tensor_add(rsum_sb[qt], hsum[0], hsum[1])
                    nc.vector.reciprocal(rsum_sb[qt], rsum_sb[qt])

                    probsT = s3.tile([128, S], BF16, tag="probsT", bufs=2)
                    for g2 in range(2):
                        tp = pssm.tile([128, 1024], BF16, tag="tp")
                        for j in range(8):
                            kt = g2 * 8 + j
                            nc.tensor.transpose(
                                tp[:, j * 128:(j + 1) * 128],
                                probs[:, kt * 128:(kt + 1) * 128],
                                identity,
                            )
                        nc.vector.tensor_copy(
                            out=probsT[:, g2 * 1024:(g2 + 1) * 1024], in_=tp
                        )

                    out_sb = s3.tile([128, H], F32, tag="out_sb", bufs=2)
                    for dc in range(NC_H):
                        ctx = pssm.tile([128, 512], F32, tag="ctx")
                        for kt in range(NT_S):
                            nc.tensor.matmul(
                                ctx,
                                probsT[:, kt * 128:(kt + 1) * 128],
                                v_big[:, kt * H + dc * 512:kt * H + (dc + 1) * 512],
                                start=(kt == 0),
                                stop=(kt == NT_S - 1),
                            )
                        nc.vector.tensor_scalar_mul(
                            out_sb[:, dc * 512:(dc + 1) * 512], ctx, rsum_sb[qt]
                        )
                        nc.sync.dma_start(
                            out=out_ext[qsl, dc * 512:(dc + 1) * 512],
                            in_=out_sb[:, dc * 512:(dc + 1) * 512],
                        )

    _split_multi_wait_instructions(nc)
    return nc


_cache = {}
last_results = None


def kernel(hidden_states, past_attention, Wq, Wk, Wv, Wm, w_gate):
    global last_results
    hs = np.asarray(hidden_states, dtype=np.float32)
    pa = np.asarray(past_attention, dtype=np.float32)
    Wq = np.asarray(Wq, dtype=np.float32)
    Wk = np.asarray(Wk, dtype=np.float32)
    Wv = np.asarray(Wv, dtype=np.float32)
    Wm = np.asarray(Wm, dtype=np.float32)
    w_gate = np.asarray(w_gate, dtype=np.float32)

    bf = ml_dtypes.bfloat16
    inv_sqrt_h = 1.0 / math.sqrt(H)
    decay = math.exp(-0.5)

    wq_b = np.ascontiguousarray(Wq).astype(bf)
    wk_s = Wk * inv_sqrt_h
    wk_half = [
        np.ascontiguousarray(wk_s[:, p * 512:(p + 1) * 512]).astype(bf)
        for p in range(2)
    ]
    wv_half = [
        np.ascontiguousarray(Wv[:, p * 512:(p + 1) * 512]).astype(bf)
        for p in range(2)
    ]
    f8 = ml_dtypes.float8_e4m3
    wm8 = np.ascontiguousarray(
        (Wm * (decay * WM_SCALE)).reshape(NT_H // 2, 2, 128, H).transpose(0, 2, 1, 3)
    ).astype(f8)
    wg_b = np.ascontiguousarray(w_gate.reshape(NT_H, 128).T).astype(bf)

    in_maps = []
    hsT_by_batch = [np.ascontiguousarray(hs[b].T).astype(bf) for b in range(B)]
    for c in range(N_CORES):
        b, h = divmod(c, 2)
        hsT = hsT_by_batch[b]  # [H, S] shared by the pair (same key order)
        hsq = hsT[:, h * SQ:(h + 1) * SQ]  # [H, SQ] this core's query rows
        paT = pa[b, h * SQ:(h + 1) * SQ, :].T  # [S, SQ]
        pa8 = np.ascontiguousarray(
            (paT * PA_SCALE).reshape(NT_S // 2, 2, 128, SQ).transpose(0, 2, 1, 3)
        ).astype(f8)
        in_maps.append(
            {
                "hsT": hsT,
                "hsq": np.ascontiguousarray(hsq),
                "pa8": pa8,
                "wq": wq_b,
                "wk": wk_half[h],
                "wv": wv_half[h],
                "wm": wm8,
                "wg": wg_b,
            }
        )

    if "nc" not in _cache:
        _cache["nc"] = build_nc()
    nc = _cache["nc"]

    res = run_bass_kernel_spmd(nc, in_maps, core_ids=list(range(N_CORES)))
    last_results = res

    out = np.empty((B, S, H), dtype=np.float32)
    for c in range(N_CORES):
        b, h = divmod(c, 2)
        out[b, h * SQ:(h + 1) * SQ, :] = res.results[c]["out"]
    return out


# revision 24
# speedup vs baseline: 1.0282x; 1.0282x over previous
"""AMRPA attention wrapper kernel for 8 TRN2 NeuronCores.

Sharding: data-parallel over (batch, seq-half). Core c handles batch b=c//2,
query rows [h*1024, (h+1)*1024) with h=c%2. Each core computes k/v for its
full batch (duplicated across the pair) and its own half of the query rows;
outputs are concatenated on host. No collectives.

SPMD trick: all cores run one graph that reads query columns [0, 1024) of
hsT; the host rolls hsT's sequence axis (and paT's key axis identically) so
each core's query rows land there. Key order is permuted consistently in
kT/v/paT, and every contraction over keys is permutation-invariant.

Math (per core, Sq=1024 query rows, S=2048 keys, H=1024):
  qT = Wq^T hs^T, kT = (Wk/sqrt(H))^T hs^T, v = hs Wv           (T-major)
  g = sigmoid(q . w_gate)                                        (per row)
  mvT = v^T paT  (contraction over keys; paT = pa^T)
  tfT = (Wm e^-0.5)^T mvT
  qhatT = qT + g * tfT      (folds the memory bias into the query)
  logits = qhat kT          (scores + memory_bias in one matmul)
  probs = exp(logits)       (logits are O(1) -- no max subtraction needed)
  context = (probs v) / rowsum(probs)

All matmul operands bf16 (fp32 PSUM accumulation); scale factors folded into
the weights on host.
"""

import math
from contextlib import ExitStack

import numpy as np
import ml_dtypes

import concourse.bass as bass
import concourse.mybir as mybir
import concourse.tile as tile
from concourse.bass_utils import run_bass_kernel_spmd
from concourse.masks import make_identity
from concourse.vector_clock import ScopedClock

BF16 = mybir.dt.bfloat16
F32 = mybir.dt.float32

B, S, H = 4, 2048, 1024
SQ = S // 2  # query rows per core
N_CORES = 8
NT_H = H // 128   # 8 partition tiles over hidden dim
NT_S = S // 128   # 16 partition tiles over sequence
NT_Q = SQ // 128  # 8 query row tiles per core
NC_S = S // 512   # 4 free-dim chunks over sequence
NC_Q = SQ // 512  # 2 free-dim chunks over query rows
NC_H = H // 512   # 2 free-dim chunks over hidden

# ---------------------------------------------------------------------------
# Workaround: this walrus build allows only one sync-wait on a Drain
# instruction; Tile's kernel-tail drain carries one wait per DMA-HW
# semaphore. Split the tail drain into a chain of single-wait drains.
# ---------------------------------------------------------------------------


def _patched_drain_and_barrier(self, tick_clock, wait_clock):
    nc = self.nc
    drain_inst = nc.sync.drain()
    wait_clock.add_sem_waits(
        drain_inst.ins, ScopedClock({None: tick_clock.global_clock})
    )
    si = drain_inst.ins.sync_info
    if si is not None and si.on_wait and len(si.on_wait) > 1:
        waits = list(si.on_wait)
        si.on_wait = waits[:1]
        for w in waits[1:]:
            d = nc.sync.drain()
            dsi = d.ins.sync_info
            if dsi is None:
                d.ins.sync_info = mybir.SyncInfo(on_wait=[w], on_update=[])
            else:
                dsi.on_wait = [w]

    nc.all_engine_barrier()
    assert self.sems is not None
    popped = nc._tile_sem_poison_stack.pop()
    assert popped is self._sem_poison
    nc.clear_and_free_semaphores(list(self.sems.allocated().values()))
    nc.all_engine_barrier()


tile.TileContext._drain_and_barrier = _patched_drain_and_barrier


def _split_multi_wait_instructions(nc: bass.Bass):
    """Walrus here allows only one sync-wait per instruction. Move extra
    waits onto injected same-engine NoOps placed just before the owner."""
    bbs = [(bb, list(bb.instructions)) for f in nc.m.functions for bb in f.blocks]
    new_lists = []
    for bb, insts in bbs:
        new_list = []
        for inst in insts:
            si = inst.sync_info
            if si is not None and si.on_wait and len(si.on_wait) > 1:
                waits = list(si.on_wait)
                for w in waits[:-1]:
                    bi = nc.engines[inst.engine].nop(nofuse=True)
                    ni = bi.ins
                    ni.sync_info = mybir.SyncInfo(on_wait=[w], on_update=[])
                    new_list.append(ni)
                si.on_wait = [waits[-1]]
            new_list.append(inst)
        new_lists.append((bb, new_list))
    for bb, nl in new_lists:
        bb.instructions = nl


def build_nc() -> bass.Bass:
    nc = bass.Bass()

    hsT_ext = nc.declare_dram_parameter("hsT", [H, S], BF16, isOutput=False)
    hsq_ext = nc.declare_dram_parameter("hsq", [H, SQ], BF16, isOutput=False)
    paT_ext = nc.declare_dram_parameter("paT", [S, SQ], BF16, isOutput=False)
    wq_ext = nc.declare_dram_parameter("wq", [H, H], BF16, isOutput=False)
    # wk/wv carry only this core's output-column half; the other half of
    # kT / v comes from the pair AllGather below
    wk_ext = nc.declare_dram_parameter("wk", [H, H // 2], BF16, isOutput=False)
    wv_ext = nc.declare_dram_parameter("wv", [H, H // 2], BF16, isOutput=False)
    wm_ext = nc.declare_dram_parameter("wm", [H, H], BF16, isOutput=False)
    wg_ext = nc.declare_dram_parameter("wg", [128, NT_H], BF16, isOutput=False)
    out_ext = nc.declare_dram_parameter("out", [SQ, H], F32, isOutput=True)

    PAIR_GROUPS = [[2 * i, 2 * i + 1] for i in range(N_CORES // 2)]

    MULT = mybir.AluOpType.mult
    ADD = mybir.AluOpType.add

    with tile.TileContext(nc) as tc:
        with tc.tile_pool(name="persist", bufs=1) as pp:
            # small constants
            identity = pp.tile([128, 128], BF16)
            make_identity(nc, identity)
            ones_row = pp.tile([1, 128], BF16)
            nc.vector.memset(ones_row, 1.0)
            wg_sb = pp.tile([128, NT_H], BF16)
            nc.sync.dma_start(out=wg_sb, in_=wg_ext[:, :])

            k_big = pp.tile([128, NT_H * S], BF16)
            k3d = k_big.rearrange("p (t s) -> p t s", s=S)
            qT_sb = [pp.tile([128, SQ], BF16, name=f"qT{t}") for t in range(NT_H)]
            v_big = pp.tile([128, NT_S * H], BF16)
            v3d = v_big.rearrange("p (st d) -> p st d", d=H)
            pa_pre = [pp.tile([128, SQ], BF16, name=f"papre{t}") for t in range(NT_S // 2)]
            qhatT_sb = [
                pp.tile([128, SQ], BF16, name=f"qhatT{t}") for t in range(NT_H)
            ]
            g_bcast = pp.tile([128, SQ], BF16)
            g_row = pp.tile([1, SQ], BF16)
            rsum_sb = [pp.tile([128, 1], F32, name=f"rsum{t}") for t in range(NT_Q)]

            # ---- stage 1: projections (kT, qT, v) + gates ----
            # Each core computes the d-half of v and the ho-half of kT its
            # wv/wk shard selects (even core: logical first half, odd core:
            # second half), always into the FIRST-half slots; the pair
            # AllGather then fills both halves in logical rank order.
            _ps_stack = ExitStack()
            ps1 = _ps_stack.enter_context(
                tc.tile_pool(name="ps12", bufs=4, space="PSUM")
            )
            with (
                tc.tile_pool(name="stage1", bufs=1) as s1,
                tc.tile_pool(name="dram_cc", bufs=1, space="DRAM") as dcc,
            ):
                hsT_sb = [
                    s1.tile([128, S], BF16, name=f"hsT{t}") for t in range(NT_H)
                ]
                wq_sb = [
                    s1.tile([128, H], BF16, name=f"wqs{t}") for t in range(NT_H)
                ]
                wk_sb = [
                    s1.tile([128, H // 2], BF16, name=f"wks{t}") for t in range(NT_H)
                ]
                wv_sb = [
                    s1.tile([128, H // 2], BF16, name=f"wvs{t}") for t in range(NT_H)
                ]
                hsq_sb = [
                    s1.tile([128, SQ], BF16, name=f"hsq{t}") for t in range(NT_H)
                ]
                # hsT + wv first: the v matmuls are the first consumers
                for t in range(NT_H):
                    nc.sync.dma_start(out=hsT_sb[t], in_=hsT_ext[t * 128:(t + 1) * 128, :])
                    nc.sync.dma_start(out=wv_sb[t], in_=wv_ext[t * 128:(t + 1) * 128, :])
                for t in range(NT_H):
                    nc.sync.dma_start(out=wk_sb[t], in_=wk_ext[t * 128:(t + 1) * 128, :])
                for t in range(NT_H):
                    nc.sync.dma_start(out=wq_sb[t], in_=wq_ext[t * 128:(t + 1) * 128, :])
                    nc.sync.dma_start(out=hsq_sb[t], in_=hsq_ext[t * 128:(t + 1) * 128, :])
                for t in range(NT_S // 2):
                    nc.sync.dma_start(out=pa_pre[t], in_=paT_ext[t * 128:(t + 1) * 128, :])

                vb_in = dcc.tile([S, H // 2], BF16)
                vb_out = dcc.tile([2 * S, H // 2], BF16)
                kb_in = dcc.tile([H // 2, S], BF16)
                kb_out = dcc.tile([H, S], BF16)

                # v half: v[s, d_local] = sum_hi hsT[hi, s] Wv_half[hi, d_local]
                for st in range(NT_S):
                    acc = ps1.tile([128, 512], F32, tag="acc")
                    for hi in range(NT_H):
                        nc.tensor.matmul(
                            acc,
                            hsT_sb[hi][:, st * 128:(st + 1) * 128],
                            wv_sb[hi],
                            start=(hi == 0),
                            stop=(hi == NT_H - 1),
                        )
                    nc.any.tensor_copy(out=v_big[:, st * H:st * H + 512], in_=acc)
                nc.gpsimd.dma_start(
                    out=vb_in.rearrange("(st p) d -> p st d", p=128),
                    in_=v3d[:, :, 0:512],
                )
                nc.gpsimd.collective_compute(
                    "AllGather",
                    mybir.AluOpType.bypass,
                    replica_groups=PAIR_GROUPS,
                    ins=[vb_in.opt()],
                    outs=[vb_out.opt()],
                )
                nc.gpsimd.dma_start(
                    out=v3d[:, :, 0:512],
                    in_=vb_out[0:S].rearrange("(st p) d -> p st d", p=128),
                )
                nc.gpsimd.dma_start(
                    out=v3d[:, :, 512:1024],
                    in_=vb_out[S:2 * S].rearrange("(st p) d -> p st d", p=128),
                )

                # kT half: kT[ho_local, s] = sum_hi Wk_half[hi, ho_local] hsT[hi, s]
                for ho in range(NT_H // 2):
                    for sc in range(NC_S):
                        acc = ps1.tile([128, 512], F32, tag="acc")
                        for hi in range(NT_H):
                            nc.tensor.matmul(
                                acc,
                                wk_sb[hi][:, ho * 128:(ho + 1) * 128],
                                hsT_sb[hi][:, sc * 512:(sc + 1) * 512],
                                start=(hi == 0),
                                stop=(hi == NT_H - 1),
                            )
                        nc.any.tensor_copy(
                            out=k_big[:, ho * S + sc * 512:ho * S + (sc + 1) * 512],
                            in_=acc,
                        )
                nc.gpsimd.dma_start(
                    out=kb_in.rearrange("(t p) s -> p t s", p=128),
                    in_=k3d[:, 0:NT_H // 2, :],
                )
                nc.gpsimd.collective_compute(
                    "AllGather",
                    mybir.AluOpType.bypass,
                    replica_groups=PAIR_GROUPS,
                    ins=[kb_in.opt()],
                    outs=[kb_out.opt()],
                )
                nc.gpsimd.dma_start(
                    out=k3d,
                    in_=kb_out.rearrange("(t p) s -> p t s", p=128),
                )

                # qT over this core's query rows (hsq input)
                for ho in range(NT_H):
                    for qc in range(NC_Q):
                        acc = ps1.tile([128, 512], F32, tag="acc")
                        for hi in range(NT_H):
                            nc.tensor.matmul(
                                acc,
                                wq_sb[hi][:, ho * 128:(ho + 1) * 128],
                                hsq_sb[hi][:, qc * 512:(qc + 1) * 512],
                                start=(hi == 0),
                                stop=(hi == NT_H - 1),
                            )
                        nc.any.tensor_copy(out=qT_sb[ho][:, qc * 512:(qc + 1) * 512], in_=acc)

                # gates: glog[1, q] = sum_t wg[:, t]^T qT[t]; sigmoid; broadcast
                for qc in range(NC_Q):
                    gacc = ps1.tile([1, 512], F32, tag="gacc", bufs=2)
                    for t in range(NT_H):
                        nc.tensor.matmul(
                            gacc,
                            wg_sb[:, t:t + 1],
                            qT_sb[t][:, qc * 512:(qc + 1) * 512],
                            start=(t == 0),
                            stop=(t == NT_H - 1),
                        )
                    nc.scalar.activation(
                        g_row[:, qc * 512:(qc + 1) * 512],
                        gacc,
                        mybir.ActivationFunctionType.Sigmoid,
                    )
                for qc in range(NC_Q):
                    gb = ps1.tile([128, 512], F32, tag="gb", bufs=2)
                    nc.tensor.matmul(
                        gb,
                        ones_row,
                        g_row[:, qc * 512:(qc + 1) * 512],
                        start=True,
                        stop=True,
                    )
                    nc.any.tensor_copy(out=g_bcast[:, qc * 512:(qc + 1) * 512], in_=gb)

            # ---- stage 2: mvT, tfT, qhatT ----
            with tc.tile_pool(name="stage2", bufs=1) as s2:
                wm_sb = [
                    s2.tile([128, H], BF16, name=f"wms{t}") for t in range(NT_H)
                ]
                mvT_sb = [
                    s2.tile([128, SQ], BF16, name=f"mvT{t}") for t in range(NT_H)
                ]
                pa_sb = list(pa_pre) + [
                    s2.tile([128, SQ], BF16, name=f"pa{t}") for t in range(NT_S // 2, NT_S)
                ]
                for t in range(NT_S // 2, NT_S):
                    nc.sync.dma_start(out=pa_sb[t], in_=paT_ext[t * 128:(t + 1) * 128, :])
                for t in range(NT_H):
                    nc.sync.dma_start(out=wm_sb[t], in_=wm_ext[t * 128:(t + 1) * 128, :])

                # mvT[d, q] = sum_k v[k, d] paT[k, q]
                for qc in range(NC_Q):
                    for d in range(NT_H):
                        acc = ps1.tile([128, 512], F32, tag="acc")
                        for kt in range(NT_S):
                            nc.tensor.matmul(
                                acc,
                                v_big[:, kt * H + d * 128:kt * H + (d + 1) * 128],
                                pa_sb[kt][:, qc * 512:(qc + 1) * 512],
                                start=(kt == 0),
                                stop=(kt == NT_S - 1),
                            )
                        nc.any.tensor_copy(out=mvT_sb[d][:, qc * 512:(qc + 1) * 512], in_=acc)

                # tfT[do, q] = sum_d Wm[d, do] mvT[d, q];
                # qhatT = qT + g_bcast * tfT
                for qc in range(NC_Q):
                    for do in range(NT_H):
                        acc = ps1.tile([128, 512], F32, tag="acc")
                        for d in range(NT_H):
                            nc.tensor.matmul(
                                acc,
                                wm_sb[d][:, do * 128:(do + 1) * 128],
                                mvT_sb[d][:, qc * 512:(qc + 1) * 512],
                                start=(d == 0),
                                stop=(d == NT_H - 1),
                            )
                        sl = slice(qc * 512, (qc + 1) * 512)
                        tmp = s2.tile([128, 512], BF16, tag="gm_tmp", bufs=3)
                        nc.vector.tensor_tensor(tmp, acc, g_bcast[:, sl], MULT)
                        nc.vector.tensor_tensor(
                            qhatT_sb[do][:, sl], tmp, qT_sb[do][:, sl], ADD
                        )

            _ps_stack.close()

            # ---- stage 3: per q-tile attention ----
            # logits computed in two [128,1024] halves (2 PSUM banks each,
            # double-buffered) so exp of one half overlaps matmuls of the
            # next; exp is the only ACT-routed op here to keep its queue clear
            with (
                tc.tile_pool(name="stage3", bufs=1) as s3,
                tc.tile_pool(name="ps_logit", bufs=2, space="PSUM") as pslg,
                tc.tile_pool(name="ps_small", bufs=2, space="PSUM") as pssm,
            ):
                for qt in range(NT_Q):
                    qsl = slice(qt * 128, (qt + 1) * 128)
                    probs = s3.tile([128, S], BF16, tag="probs", bufs=2)
                    hsum = [None, None]
                    for half in range(2):
                        lg = pslg.tile([128, 1024], F32, tag="lg")
                        for kk2 in range(2):
                            kk = half * 2 + kk2
                            for d in range(NT_H):
                                nc.tensor.matmul(
                                    lg[:, kk2 * 512:(kk2 + 1) * 512],
                                    qhatT_sb[d][:, qsl],
                                    k_big[:, d * S + kk * 512:d * S + (kk + 1) * 512],
                                    start=(d == 0),
                                    stop=(d == NT_H - 1),
                                )
                        hs_t = s3.tile(
                            [128, 1], F32, tag=f"hsum{half}", bufs=2, name=f"hs{half}"
                        )
                        nc.scalar.activation(
                            probs[:, half * 1024:(half + 1) * 1024],
                            lg,
                            mybir.ActivationFunctionType.Exp,
                            accum_out=hs_t,
                        )
                        hsum[half] = hs_t
                    nc.vector.tensor_add(rsum_sb[qt], hsum[0], hsum[1])
                    nc.vector.reciprocal(rsum_sb[qt], rsum_sb[qt])

                    probsT = s3.tile([128, S], BF16, tag="probsT", bufs=2)
                    for g2 in range(2):
                        tp = pssm.tile([128, 1024], BF16, tag="tp")
                        for j in range(8):
                            kt = g2 * 8 + j
                            nc.tensor.transpose(
                                tp[:, j * 128:(j + 1) * 128],
                                probs[:, kt * 128:(kt + 1) * 128],
                                identity,
                            )
                        nc.vector.tensor_copy(
                            out=probsT[:, g2 * 1024:(g2 + 1) * 1024], in_=tp
                        )

                    out_sb = s3.tile([128, H], F32, tag="out_sb", bufs=2)
                    for dc in range(NC_H):
                        ctx = pssm.tile([128, 512], F32, tag="ctx")
                        for kt in range(NT_S):
                            nc.tensor.matmul(
                                ctx,
                                probsT[:, kt * 128:(kt + 1) * 128],
                                v_big[:, kt * H + dc * 512:kt * H + (dc + 1) * 512],
                                start=(kt == 0),
                                stop=(kt == NT_S - 1),
                            )
                        nc.vector.tensor_scalar_mul(
                            out_sb[:, dc * 512:(dc + 1) * 512], ctx, rsum_sb[qt]
                        )
                        nc.sync.dma_start(
                            out=out_ext[qsl, dc * 512:(dc + 1) * 512],
                            in_=out_sb[:, dc * 512:(dc + 1) * 512],
                        )

    _split_multi_wait_instructions(nc)
    return nc


_cache = {}
last_results = None


def kernel(hidden_states, past_attention, Wq, Wk, Wv, Wm, w_gate):
    global last_results
    hs = np.asarray(hidden_states, dtype=np.float32)
    pa = np.asarray(past_attention, dtype=np.float32)
    Wq = np.asarray(Wq, dtype=np.float32)
    Wk = np.asarray(Wk, dtype=np.float32)
    Wv = np.asarray(Wv, dtype=np.float32)
    Wm = np.asarray(Wm, dtype=np.float32)
    w_gate = np.asarray(w_gate, dtype=np.float32)

    bf = ml_dtypes.bfloat16
    inv_sqrt_h = 1.0 / math.sqrt(H)
    decay = math.exp(-0.5)

    wq_b = np.ascontiguousarray(Wq).astype(bf)
    wk_s = Wk * inv_sqrt_h
    wk_half = [
        np.ascontiguousarray(wk_s[:, p * 512:(p + 1) * 512]).astype(bf)
        for p in range(2)
    ]
    wv_half = [
        np.ascontiguousarray(Wv[:, p * 512:(p + 1) * 512]).astype(bf)
        for p in range(2)
    ]
    wm_b = np.ascontiguousarray(Wm * decay).astype(bf)
    wg_b = np.ascontiguousarray(w_gate.reshape(NT_H, 128).T).astype(bf)

    in_maps = []
    hsT_by_batch = [np.ascontiguousarray(hs[b].T).astype(bf) for b in range(B)]
    for c in range(N_CORES):
        b, h = divmod(c, 2)
        hsT = hsT_by_batch[b]  # [H, S] shared by the pair (same key order)
        hsq = hsT[:, h * SQ:(h + 1) * SQ]  # [H, SQ] this core's query rows
        paT = pa[b, h * SQ:(h + 1) * SQ, :].T  # [S, SQ]
        in_maps.append(
            {
                "hsT": hsT,
                "hsq": np.ascontiguousarray(hsq),
                "paT": np.ascontiguousarray(paT).astype(bf),
                "wq": wq_b,
                "wk": wk_half[h],
                "wv": wv_half[h],
                "wm": wm_b,
                "wg": wg_b,
            }
        )

    if "nc" not in _cache:
        _cache["nc"] = build_nc()
    nc = _cache["nc"]

    res = run_bass_kernel_spmd(nc, in_maps, core_ids=list(range(N_CORES)))
    last_results = res

    out = np.empty((B, S, H), dtype=np.float32)
    for c in range(N_CORES):
        b, h = divmod(c, 2)
        out[b, h * SQ:(h + 1) * SQ, :] = res.results[c]["out"]
    return out


# revision 25
# speedup vs baseline: 1.0455x; 1.0168x over previous
"""AMRPA attention wrapper kernel for 8 TRN2 NeuronCores.

Sharding: data-parallel over (batch, seq-half). Core c handles batch b=c//2,
query rows [h*1024, (h+1)*1024) with h=c%2. Each core computes k/v for its
full batch (duplicated across the pair) and its own half of the query rows;
outputs are concatenated on host. No collectives.

SPMD trick: all cores run one graph that reads query columns [0, 1024) of
hsT; the host rolls hsT's sequence axis (and paT's key axis identically) so
each core's query rows land there. Key order is permuted consistently in
kT/v/paT, and every contraction over keys is permutation-invariant.

Math (per core, Sq=1024 query rows, S=2048 keys, H=1024):
  qT = Wq^T hs^T, kT = (Wk/sqrt(H))^T hs^T, v = hs Wv           (T-major)
  g = sigmoid(q . w_gate)                                        (per row)
  mvT = v^T paT  (contraction over keys; paT = pa^T)
  tfT = (Wm e^-0.5)^T mvT
  qhatT = qT + g * tfT      (folds the memory bias into the query)
  logits = qhat kT          (scores + memory_bias in one matmul)
  probs = exp(logits)       (logits are O(1) -- no max subtraction needed)
  context = (probs v) / rowsum(probs)

All matmul operands bf16 (fp32 PSUM accumulation); scale factors folded into
the weights on host.
"""

import math

import numpy as np
import ml_dtypes

import concourse.bass as bass
import concourse.mybir as mybir
import concourse.tile as tile
from concourse.bass_utils import run_bass_kernel_spmd
from concourse.masks import make_identity
from concourse.vector_clock import ScopedClock

BF16 = mybir.dt.bfloat16
F32 = mybir.dt.float32

B, S, H = 4, 2048, 1024
SQ = S // 2  # query rows per core
N_CORES = 8
NT_H = H // 128   # 8 partition tiles over hidden dim
NT_S = S // 128   # 16 partition tiles over sequence
NT_Q = SQ // 128  # 8 query row tiles per core
NC_S = S // 512   # 4 free-dim chunks over sequence
NC_Q = SQ // 512  # 2 free-dim chunks over query rows
NC_H = H // 512   # 2 free-dim chunks over hidden

# ---------------------------------------------------------------------------
# Workaround: this walrus build allows only one sync-wait on a Drain
# instruction; Tile's kernel-tail drain carries one wait per DMA-HW
# semaphore. Split the tail drain into a chain of single-wait drains.
# ---------------------------------------------------------------------------


def _patched_drain_and_barrier(self, tick_clock, wait_clock):
    nc = self.nc
    drain_inst = nc.sync.drain()
    wait_clock.add_sem_waits(
        drain_inst.ins, ScopedClock({None: tick_clock.global_clock})
    )
    si = drain_inst.ins.sync_info
    if si is not None and si.on_wait and len(si.on_wait) > 1:
        waits = list(si.on_wait)
        si.on_wait = waits[:1]
        for w in waits[1:]:
            d = nc.sync.drain()
            dsi = d.ins.sync_info
            if dsi is None:
                d.ins.sync_info = mybir.SyncInfo(on_wait=[w], on_update=[])
            else:
                dsi.on_wait = [w]

    nc.all_engine_barrier()
    assert self.sems is not None
    popped = nc._tile_sem_poison_stack.pop()
    assert popped is self._sem_poison
    nc.clear_and_free_semaphores(list(self.sems.allocated().values()))
    nc.all_engine_barrier()


tile.TileContext._drain_and_barrier = _patched_drain_and_barrier


def _split_multi_wait_instructions(nc: bass.Bass):
    """Walrus here allows only one sync-wait per instruction. Move extra
    waits onto injected same-engine NoOps placed just before the owner."""
    bbs = [(bb, list(bb.instructions)) for f in nc.m.functions for bb in f.blocks]
    new_lists = []
    for bb, insts in bbs:
        new_list = []
        for inst in insts:
            si = inst.sync_info
            if si is not None and si.on_wait and len(si.on_wait) > 1:
                waits = list(si.on_wait)
                for w in waits[:-1]:
                    bi = nc.engines[inst.engine].nop(nofuse=True)
                    ni = bi.ins
                    ni.sync_info = mybir.SyncInfo(on_wait=[w], on_update=[])
                    new_list.append(ni)
                si.on_wait = [waits[-1]]
            new_list.append(inst)
        new_lists.append((bb, new_list))
    for bb, nl in new_lists:
        bb.instructions = nl


def build_nc() -> bass.Bass:
    nc = bass.Bass()

    hsT_ext = nc.declare_dram_parameter("hsT", [H, S], BF16, isOutput=False)
    hsq_ext = nc.declare_dram_parameter("hsq", [H, SQ], BF16, isOutput=False)
    paT_ext = nc.declare_dram_parameter("paT", [S, SQ], BF16, isOutput=False)
    wq_ext = nc.declare_dram_parameter("wq", [H, H], BF16, isOutput=False)
    # wk/wv carry only this core's output-column half; the other half of
    # kT / v comes from the pair AllGather below
    wk_ext = nc.declare_dram_parameter("wk", [H, H // 2], BF16, isOutput=False)
    wv_ext = nc.declare_dram_parameter("wv", [H, H // 2], BF16, isOutput=False)
    wm_ext = nc.declare_dram_parameter("wm", [H, H], BF16, isOutput=False)
    wg_ext = nc.declare_dram_parameter("wg", [128, NT_H], BF16, isOutput=False)
    out_ext = nc.declare_dram_parameter("out", [SQ, H], F32, isOutput=True)

    PAIR_GROUPS = [[2 * i, 2 * i + 1] for i in range(N_CORES // 2)]

    MULT = mybir.AluOpType.mult
    ADD = mybir.AluOpType.add

    with tile.TileContext(nc) as tc:
        with tc.tile_pool(name="persist", bufs=1) as pp:
            # small constants
            identity = pp.tile([128, 128], BF16)
            make_identity(nc, identity)
            ones_row = pp.tile([1, 128], BF16)
            nc.vector.memset(ones_row, 1.0)
            wg_sb = pp.tile([128, NT_H], BF16)
            nc.sync.dma_start(out=wg_sb, in_=wg_ext[:, :])

            kT_sb = [pp.tile([128, S], BF16, name=f"kT{t}") for t in range(NT_H)]
            qT_sb = [pp.tile([128, SQ], BF16, name=f"qT{t}") for t in range(NT_H)]
            v_sb = [pp.tile([128, H], BF16, name=f"v{t}") for t in range(NT_S)]
            pa_pre = [pp.tile([128, SQ], BF16, name=f"papre{t}") for t in range(NT_S // 2)]
            qhatT_sb = [
                pp.tile([128, SQ], BF16, name=f"qhatT{t}") for t in range(NT_H)
            ]
            g_bcast = pp.tile([128, SQ], BF16)
            g_row = pp.tile([1, SQ], BF16)
            rsum_sb = [pp.tile([128, 1], F32, name=f"rsum{t}") for t in range(NT_Q)]

            # ---- stage 1: projections (kT, qT, v) + gates ----
            # Each core computes the d-half of v and the ho-half of kT its
            # wv/wk shard selects (even core: logical first half, odd core:
            # second half), always into the FIRST-half slots; the pair
            # AllGather then fills both halves in logical rank order.
            with (
                tc.tile_pool(name="stage1", bufs=1) as s1,
                tc.tile_pool(name="dram_cc", bufs=1, space="DRAM") as dcc,
                tc.tile_pool(name="ps1", bufs=4, space="PSUM") as ps1,
            ):
                hsT_sb = [
                    s1.tile([128, S], BF16, name=f"hsT{t}") for t in range(NT_H)
                ]
                wq_sb = [
                    s1.tile([128, H], BF16, name=f"wqs{t}") for t in range(NT_H)
                ]
                wk_sb = [
                    s1.tile([128, H // 2], BF16, name=f"wks{t}") for t in range(NT_H)
                ]
                wv_sb = [
                    s1.tile([128, H // 2], BF16, name=f"wvs{t}") for t in range(NT_H)
                ]
                hsq_sb = [
                    s1.tile([128, SQ], BF16, name=f"hsq{t}") for t in range(NT_H)
                ]
                # hsT + wv first: the v matmuls are the first consumers
                for t in range(NT_H):
                    nc.sync.dma_start(out=hsT_sb[t], in_=hsT_ext[t * 128:(t + 1) * 128, :])
                    nc.sync.dma_start(out=wv_sb[t], in_=wv_ext[t * 128:(t + 1) * 128, :])
                for t in range(NT_H):
                    nc.sync.dma_start(out=wk_sb[t], in_=wk_ext[t * 128:(t + 1) * 128, :])
                for t in range(NT_H):
                    nc.sync.dma_start(out=wq_sb[t], in_=wq_ext[t * 128:(t + 1) * 128, :])
                    nc.sync.dma_start(out=hsq_sb[t], in_=hsq_ext[t * 128:(t + 1) * 128, :])
                for t in range(NT_S // 2):
                    nc.sync.dma_start(out=pa_pre[t], in_=paT_ext[t * 128:(t + 1) * 128, :])

                vb_in = dcc.tile([S, H // 2], BF16)
                vb_out = dcc.tile([2 * S, H // 2], BF16)
                kb_in = dcc.tile([H // 2, S], BF16)
                kb_out = dcc.tile([H, S], BF16)

                # v half: v[s, d_local] = sum_hi hsT[hi, s] Wv_half[hi, d_local]
                for st in range(NT_S):
                    acc = ps1.tile([128, 512], F32, tag="acc")
                    for hi in range(NT_H):
                        nc.tensor.matmul(
                            acc,
                            hsT_sb[hi][:, st * 128:(st + 1) * 128],
                            wv_sb[hi],
                            start=(hi == 0),
                            stop=(hi == NT_H - 1),
                        )
                    nc.any.tensor_copy(out=v_sb[st][:, 0:512], in_=acc)
                    nc.gpsimd.dma_start(
                        out=vb_in[st * 128:(st + 1) * 128, :], in_=v_sb[st][:, 0:512]
                    )
                nc.gpsimd.collective_compute(
                    "AllGather",
                    mybir.AluOpType.bypass,
                    replica_groups=PAIR_GROUPS,
                    ins=[vb_in.opt()],
                    outs=[vb_out.opt()],
                )
                for st in range(NT_S):
                    nc.gpsimd.dma_start(
                        out=v_sb[st][:, 0:512],
                        in_=vb_out[st * 128:(st + 1) * 128, :],
                    )
                    nc.gpsimd.dma_start(
                        out=v_sb[st][:, 512:1024],
                        in_=vb_out[S + st * 128:S + (st + 1) * 128, :],
                    )

                # kT half: kT[ho_local, s] = sum_hi Wk_half[hi, ho_local] hsT[hi, s]
                for ho in range(NT_H // 2):
                    for sc in range(NC_S):
                        acc = ps1.tile([128, 512], F32, tag="acc")
                        for hi in range(NT_H):
                            nc.tensor.matmul(
                                acc,
                                wk_sb[hi][:, ho * 128:(ho + 1) * 128],
                                hsT_sb[hi][:, sc * 512:(sc + 1) * 512],
                                start=(hi == 0),
                                stop=(hi == NT_H - 1),
                            )
                        nc.any.tensor_copy(out=kT_sb[ho][:, sc * 512:(sc + 1) * 512], in_=acc)
                for t in range(NT_H // 2):
                    nc.gpsimd.dma_start(
                        out=kb_in[t * 128:(t + 1) * 128, :], in_=kT_sb[t]
                    )
                nc.gpsimd.collective_compute(
                    "AllGather",
                    mybir.AluOpType.bypass,
                    replica_groups=PAIR_GROUPS,
                    ins=[kb_in.opt()],
                    outs=[kb_out.opt()],
                )
                for t in range(NT_H):
                    nc.gpsimd.dma_start(
                        out=kT_sb[t], in_=kb_out[t * 128:(t + 1) * 128, :]
                    )

                # qT over this core's query rows (hsq input)
                for ho in range(NT_H):
                    for qc in range(NC_Q):
                        acc = ps1.tile([128, 512], F32, tag="acc")
                        for hi in range(NT_H):
                            nc.tensor.matmul(
                                acc,
                                wq_sb[hi][:, ho * 128:(ho + 1) * 128],
                                hsq_sb[hi][:, qc * 512:(qc + 1) * 512],
                                start=(hi == 0),
                                stop=(hi == NT_H - 1),
                            )
                        nc.any.tensor_copy(out=qT_sb[ho][:, qc * 512:(qc + 1) * 512], in_=acc)

                # gates: glog[1, q] = sum_t wg[:, t]^T qT[t]; sigmoid; broadcast
                for qc in range(NC_Q):
                    gacc = ps1.tile([1, 512], F32, tag="gacc", bufs=2)
                    for t in range(NT_H):
                        nc.tensor.matmul(
                            gacc,
                            wg_sb[:, t:t + 1],
                            qT_sb[t][:, qc * 512:(qc + 1) * 512],
                            start=(t == 0),
                            stop=(t == NT_H - 1),
                        )
                    nc.scalar.activation(
                        g_row[:, qc * 512:(qc + 1) * 512],
                        gacc,
                        mybir.ActivationFunctionType.Sigmoid,
                    )
                for qc in range(NC_Q):
                    gb = ps1.tile([128, 512], F32, tag="gb", bufs=2)
                    nc.tensor.matmul(
                        gb,
                        ones_row,
                        g_row[:, qc * 512:(qc + 1) * 512],
                        start=True,
                        stop=True,
                    )
                    nc.any.tensor_copy(out=g_bcast[:, qc * 512:(qc + 1) * 512], in_=gb)

            # ---- stage 2: mvT, tfT, qhatT ----
            with (
                tc.tile_pool(name="stage2", bufs=1) as s2,
                tc.tile_pool(name="ps2", bufs=6, space="PSUM") as ps2,
            ):
                wm_sb = [
                    s2.tile([128, H], BF16, name=f"wms{t}") for t in range(NT_H)
                ]
                mvT_sb = [
                    s2.tile([128, SQ], BF16, name=f"mvT{t}") for t in range(NT_H)
                ]
                pa_sb = list(pa_pre) + [
                    s2.tile([128, SQ], BF16, name=f"pa{t}") for t in range(NT_S // 2, NT_S)
                ]
                for t in range(NT_S // 2, NT_S):
                    nc.sync.dma_start(out=pa_sb[t], in_=paT_ext[t * 128:(t + 1) * 128, :])
                for t in range(NT_H):
                    nc.sync.dma_start(out=wm_sb[t], in_=wm_ext[t * 128:(t + 1) * 128, :])

                # mvT[d, q] = sum_k v[k, d] paT[k, q]
                for qc in range(NC_Q):
                    for d in range(NT_H):
                        acc = ps2.tile([128, 512], F32, tag="acc2")
                        for kt in range(NT_S):
                            nc.tensor.matmul(
                                acc,
                                v_sb[kt][:, d * 128:(d + 1) * 128],
                                pa_sb[kt][:, qc * 512:(qc + 1) * 512],
                                start=(kt == 0),
                                stop=(kt == NT_S - 1),
                            )
                        nc.any.tensor_copy(out=mvT_sb[d][:, qc * 512:(qc + 1) * 512], in_=acc)

                # tfT[do, q] = sum_d Wm[d, do] mvT[d, q];
                # qhatT = qT + g_bcast * tfT
                for qc in range(NC_Q):
                    for do in range(NT_H):
                        acc = ps2.tile([128, 512], F32, tag="acc2")
                        for d in range(NT_H):
                            nc.tensor.matmul(
                                acc,
                                wm_sb[d][:, do * 128:(do + 1) * 128],
                                mvT_sb[d][:, qc * 512:(qc + 1) * 512],
                                start=(d == 0),
                                stop=(d == NT_H - 1),
                            )
                        sl = slice(qc * 512, (qc + 1) * 512)
                        tmp = s2.tile([128, 512], BF16, tag="gm_tmp", bufs=3)
                        nc.vector.tensor_tensor(tmp, acc, g_bcast[:, sl], MULT)
                        nc.vector.tensor_tensor(
                            qhatT_sb[do][:, sl], tmp, qT_sb[do][:, sl], ADD
                        )

            # ---- stage 3: per q-tile attention ----
            # logits computed in two [128,1024] halves (2 PSUM banks each,
            # double-buffered) so exp of one half overlaps matmuls of the
            # next; exp is the only ACT-routed op here to keep its queue clear
            with (
                tc.tile_pool(name="stage3", bufs=1) as s3,
                tc.tile_pool(name="ps_logit", bufs=2, space="PSUM") as pslg,
                tc.tile_pool(name="ps_small", bufs=2, space="PSUM") as pssm,
            ):
                for qt in range(NT_Q):
                    qsl = slice(qt * 128, (qt + 1) * 128)
                    probs = s3.tile([128, S], BF16, tag="probs", bufs=2)
                    hsum = [None, None]
                    for half in range(2):
                        lg = pslg.tile([128, 1024], F32, tag="lg")
                        for kk2 in range(2):
                            kk = half * 2 + kk2
                            for d in range(NT_H):
                                nc.tensor.matmul(
                                    lg[:, kk2 * 512:(kk2 + 1) * 512],
                                    qhatT_sb[d][:, qsl],
                                    kT_sb[d][:, kk * 512:(kk + 1) * 512],
                                    start=(d == 0),
                                    stop=(d == NT_H - 1),
                                )
                        hs_t = s3.tile(
                            [128, 1], F32, tag=f"hsum{half}", bufs=2, name=f"hs{half}"
                        )
                        nc.scalar.activation(
                            probs[:, half * 1024:(half + 1) * 1024],
                            lg,
                            mybir.ActivationFunctionType.Exp,
                            accum_out=hs_t,
                        )
                        hsum[half] = hs_t
                    nc.vector.tensor_add(rsum_sb[qt], hsum[0], hsum[1])
                    nc.vector.reciprocal(rsum_sb[qt], rsum_sb[qt])

                    probsT = s3.tile([128, S], BF16, tag="probsT", bufs=2)
                    for g2 in range(2):
                        tp = pssm.tile([128, 1024], BF16, tag="tp")
                        for j in range(8):
                            kt = g2 * 8 + j
                            nc.tensor.transpose(
                                tp[:, j * 128:(j + 1) * 128],
                                probs[:, kt * 128:(kt + 1) * 128],
                                identity,
                            )
                        nc.vector.tensor_copy(
                            out=probsT[:, g2 * 1024:(g2 + 1) * 1024], in_=tp
                        )

                    out_sb = s3.tile([128, H], F32, tag="out_sb", bufs=2)
                    for dc in range(NC_H):
                        ctx = pssm.tile([128, 512], F32, tag="ctx")
                        for kt in range(NT_S):
                            nc.tensor.matmul(
                                ctx,
                                probsT[:, kt * 128:(kt + 1) * 128],
                                v_sb[kt][:, dc * 512:(dc + 1) * 512],
                                start=(kt == 0),
                                stop=(kt == NT_S - 1),
                            )
                        nc.vector.tensor_scalar_mul(
                            out_sb[:, dc * 512:(dc + 1) * 512], ctx, rsum_sb[qt]
                        )
                        nc.sync.dma_start(
                            out=out_ext[qsl, dc * 512:(dc + 1) * 512],
                            in_=out_sb[:, dc * 512:(dc + 1) * 512],
                        )

    _split_multi_wait_instructions(nc)
    return nc


_cache = {}
last_results = None


def kernel(hidden_states, past_attention, Wq, Wk, Wv, Wm, w_gate):
    global last_results
    hs = np.asarray(hidden_states, dtype=np.float32)
    pa = np.asarray(past_attention, dtype=np.float32)
    Wq = np.asarray(Wq, dtype=np.float32)
    Wk = np.asarray(Wk, dtype=np.float32)
    Wv = np.asarray(Wv, dtype=np.float32)
    Wm = np.asarray(Wm, dtype=np.float32)
    w_gate = np.asarray(w_gate, dtype=np.float32)

    bf = ml_dtypes.bfloat16
    inv_sqrt_h = 1.0 / math.sqrt(H)
    decay = math.exp(-0.5)

    wq_b = np.ascontiguousarray(Wq).astype(bf)
    wk_s = Wk * inv_sqrt_h
    wk_half = [
        np.ascontiguousarray(wk_s[:, p * 512:(p + 1) * 512]).astype(bf)
        for p in range(2)
    ]
    wv_half = [
        np.ascontiguousarray(Wv[:, p * 512:(p + 1) * 512]).astype(bf)
        for p in range(2)
    ]
    wm_b = np.ascontiguousarray(Wm * decay).astype(bf)
    wg_b = np.ascontiguousarray(w_gate.reshape(NT_H, 128).T).astype(bf)

    in_maps = []
    hsT_by_batch = [np.ascontiguousarray(hs[b].T).astype(bf) for b in range(B)]
    for c in range(N_CORES):
        b, h = divmod(c, 2)
        hsT = hsT_by_batch[b]  # [H, S] shared by the pair (same key order)
        hsq = hsT[:, h * SQ:(h + 1) * SQ]  # [H, SQ] this core's query rows
        paT = pa[b, h * SQ:(h + 1) * SQ, :].T  # [S, SQ]
        in_maps.append(
            {
                "hsT": hsT,
                "hsq": np.ascontiguousarray(hsq),
                "paT": np.ascontiguousarray(paT).astype(bf),
                "wq": wq_b,
                "wk": wk_half[h],
                "wv": wv_half[h],
                "wm": wm_b,
                "wg": wg_b,
            }
        )

    if "nc" not in _cache:
        _cache["nc"] = build_nc()
    nc = _cache["nc"]

    res = run_bass_kernel_spmd(nc, in_maps, core_ids=list(range(N_CORES)))
    last_results = res

    out = np.empty((B, S, H), dtype=np.float32)
    for c in range(N_CORES):
        b, h = divmod(c, 2)
        out[b, h * SQ:(h + 1) * SQ, :] = res.results[c]["out"]
    return out
